# revision 17
# baseline (speedup 1.0000x reference)
"""Trainium2 Bass kernel for nn_Attention_11192684774105.

Reference computation (B=4, C=128, H=W=64, N=4096, 2 heads, key_dim=32,
head_dim=64):
    qkv  = conv1x1(x, w_qkv)                    # [B,256,H,W]
    q,k,v split per head; attn = softmax(q^T k / sqrt(32)) over keys
    out  = v @ attn^T  (+ depthwise3x3(v, w_pe)) -> conv1x1(w_proj)

Sharding: 8 cores = (batch b, row-half) pairs.  Each core computes both
heads for 2048 query positions (32 image rows) of one sample; keys/values
span the full 4096 positions.  Output is a pure concatenation.

Per-core algorithm (all matmuls on PE in float32r):
  - K/V are computed from a row-rotated copy of x so the depthwise-conv
    halo rows sit at fixed positions (softmax/attention are invariant to
    a consistent permutation of the key axis).
  - S^T[m,n] = K^T Q is computed with m (key) on partitions, n in free dim,
    exp() runs on the scalar engine (no max subtraction needed: |logits|
    < 3), and the softmax denominator comes from an extra ones-column in
    the V^T stationary operand of the second matmul (row 64 of the PSUM
    accumulator = sum_m exp).
  - The depthwise conv + output projection are fused into 10 accumulating
    1x1 matmuls: W_proj @ attn_out + sum_s (W_proj * w_pe[:,s]) @ shift_s(V).
"""

import os

import numpy as np

import concourse.bass as bass
import concourse.mybir as mybir
import concourse.tile as tile
from concourse import bacc
from concourse.bass_utils import run_bass_kernel_spmd

F32 = mybir.dt.float32
F32R = mybir.dt.float32r
AF = mybir.ActivationFunctionType

B, C, H, W = 4, 128, 64, 64
N = H * W                    # 4096
NHALF = N // 2               # 2048 query positions per core
SCALE = 32 ** (-0.5)
NCHUNK = 512                 # matmul moving free dim / psum bank
NCH = NHALF // NCHUNK        # 4 chunks per core
MB = N // 128                # 32 key blocks of 128
EXP_W = 512                  # free width of each scalar-engine exp op
SBUFS = 4 if EXP_W == NCHUNK else 2   # S-psum slots (x EXP_W//512 banks)

# wpack column-block indices (each block is 128 cols)
WQ0, WQ1, WK0, WK1, WV01, WPROJT, MS0, IDENT = 0, 1, 2, 3, 4, 5, 6, 15
NWCOL = 16 * 128 + 4   # + halo_top, halo_bot, const-1.0, const-0.0 cols


def _emit_kernel(tc, io):
    """Emit the per-core program. io: dict of DRAM APs."""
    nc = tc.nc
    ctx_pools = {}

    const = tc.alloc_tile_pool(name="const", bufs=1)
    epool = tc.alloc_tile_pool(name="epool", bufs=8)
    npool = tc.alloc_tile_pool(name="npool", bufs=2)
    psum = tc.alloc_tile_pool(name="psum", bufs=1, space="PSUM")
    ctx_pools.update(const=const, epool=epool, npool=npool, psum=psum)

    # ---- load inputs (chunked so consumers start early) ------------------
    wp = const.tile([128, NWCOL], F32R)
    # qkv weight blocks first, then identity/halo/const cols, proj block last
    nc.sync.dma_start(out=wp[:, 0:5 * 128], in_=io["wpack"][:, 0:5 * 128])
    nc.sync.dma_start(out=wp[:, 15 * 128:NWCOL],
                      in_=io["wpack"][:, 15 * 128:NWCOL])
    nc.sync.dma_start(out=wp[:, 5 * 128:15 * 128],
                      in_=io["wpack"][:, 5 * 128:15 * 128])
    xr = const.tile([128, N], F32R)
    xq = const.tile([128, NHALF], F32R)
    for j in range(4):
        nc.sync.dma_start(out=xr[:, j * 1024:(j + 1) * 1024],
                          in_=io["x_rot"][:, j * 1024:(j + 1) * 1024])
    for j in range(2):
        nc.sync.dma_start(out=xq[:, j * 1024:(j + 1) * 1024],
                          in_=io["x_q"][:, j * 1024:(j + 1) * 1024])

    # ---- persistent sbuf -------------------------------------------------
    krep0 = const.tile([128, N], F32R)
    krep1 = const.tile([128, N], F32R)
    qrep0 = const.tile([128, NHALF], F32R)
    qrep1 = const.tile([128, NHALF], F32R)
    vchan = const.tile([128, N], F32R)
    vaugT = const.tile([128, MB * 130], F32R)
    vpad = const.tile([128, 34 * 66], F32R)
    attn = const.tile([128, NHALF], F32R)
    outsb = const.tile([128, NHALF], F32)
    ones64 = const.tile([128, 64], F32R)

    one_col = wp[:, 16 * 128 + 2:16 * 128 + 3]
    zero_col = wp[:, 16 * 128 + 3:16 * 128 + 4]
    # Pin the exp table set before any ACT Copy picks a different one.
    actwarm = npool.tile([128, 1], F32, tag="actwarm", name="actwarm")
    nc.scalar.activation(out=actwarm, in_=one_col, func=AF.Exp, scale=1.0)
    # Only the aug-ones columns (64,129 mod 130) and vpad border columns
    # (0,65 mod 66) actually need filling; everything else is overwritten.
    vaug4 = vaugT.rearrange("p (mb c) -> p mb c", mb=MB, c=130)
    nc.vector.tensor_copy(out=vaug4[:, :, 64:130:65],
                          in_=one_col.broadcast_to([128, MB, 2]))
    vpadr = vpad.rearrange("p (r c) -> p r c", r=34, c=66)
    nc.vector.tensor_copy(out=vpadr[:, :, 0:66:65],
                          in_=zero_col.broadcast_to([128, 34, 2]))
    nc.vector.tensor_copy(out=ones64, in_=one_col.broadcast_to([128, 64]))

    def wblk(i):
        return wp[:, i * 128:(i + 1) * 128]

    copy_engines = [nc.vector, nc.scalar]

    # ---- phase A: qkv 1x1 conv (weights pre-transposed/replicated) -------
    qkv_specs = [
        (WK0, xr, krep0, N), (WQ0, xq, qrep0, NHALF), (WV01, xr, vchan, N),
        (WK1, xr, krep1, N), (WQ1, xq, qrep1, NHALF),
    ]
    ci = 0
    for blk, src, dst, width in qkv_specs:
        for j in range(width // NCHUNK):
            ps = psum.tile([128, NCHUNK], F32, tag="s", bufs=SBUFS,
                           name="ps_a")
            nc.tensor.matmul(
                ps, lhsT=wblk(blk),
                rhs=src[:, j * NCHUNK:(j + 1) * NCHUNK],
                start=True, stop=True)
            eng = copy_engines[ci % 2]
            ci += 1
            if eng is nc.scalar:
                eng.copy(out=dst[:, j * NCHUNK:(j + 1) * NCHUNK], in_=ps)
            else:
                eng.tensor_copy(out=dst[:, j * NCHUNK:(j + 1) * NCHUNK], in_=ps)

    # ---- phase B: V^T blocks into augmented stationary layout ------------
    # vaugT block mb (130 cols): [v0^T(64) | 1 | v1^T(64) | 1]
    identity = wblk(IDENT)
    for mb in range(MB):
        pt = psum.tile([128, 128], F32R, tag="s", bufs=SBUFS, name="pt")
        nc.tensor.transpose(pt, vchan[:, mb * 128:(mb + 1) * 128], identity)
        dst = vaugT[:, mb * 130:mb * 130 + 130].rearrange(
            "p (a b) -> p a b", a=2, b=65)[:, :, 0:64]
        src = pt.rearrange("p (a b) -> p a b", a=2, b=64)
        nc.vector.tensor_copy(out=dst, in_=src)

    # ---- phase D: zero-padded V for the depthwise conv -------------------
    vpad3 = vpad.rearrange("p (r c) -> p r c", r=34, c=66)
    nc.vector.tensor_copy(out=vpad3[:, 1:33, 1:65], in_=vchan[:, 64:64 + 32 * 64])
    nc.vector.tensor_scalar(
        out=vpad3[:, 0, 1:65], in0=vchan[:, 0:64],
        scalar1=wp[:, 16 * 128:16 * 128 + 1].bitcast(F32), scalar2=None,
        op0=mybir.AluOpType.mult)
    nc.vector.tensor_scalar(
        out=vpad3[:, 33, 1:65], in0=vchan[:, 33 * 64:34 * 64],
        scalar1=wp[:, 16 * 128 + 1:16 * 128 + 2].bitcast(F32), scalar2=None,
        op0=mybir.AluOpType.mult)

    # ---- phase C: attention per head ------------------------------------
    for h in range(2):
        krep = (krep0, krep1)[h]
        qrep = (qrep0, qrep1)[h]
        oacc = []
        for c in range(NCH):
            oacc.append(psum.tile([128, NCHUNK], F32, tag="o", bufs=4,
                                  name=f"oacc{c}"))
        if EXP_W == NCHUNK:
            probe = os.environ.get("KPROBE", "")
            for g in range(MB // 4):
                for c in range(NCH):
                    etiles = []
                    for i in range(4):
                        mb = 4 * g + i
                        ii = 0 if probe == "no_tilepos" else i
                        st = psum.tile([128, NCHUNK], F32, tag="s",
                                       bufs=SBUFS, name="st")
                        if probe == "half_st" and i % 2 == 1:
                            etiles.append(etiles[-1])
                            continue
                        nc.tensor.matmul(
                            st,
                            lhsT=krep[32 * ii:32 * (ii + 1),
                                      mb * 128:(mb + 1) * 128],
                            rhs=qrep[32 * ii:32 * (ii + 1),
                                     c * NCHUNK:(c + 1) * NCHUNK],
                            start=True, stop=True, tile_position=(32 * ii, 0))
                        if probe == "half_exp" and i % 2 == 1:
                            etiles.append(etiles[-1])
                            continue
                        et = epool.tile([128, NCHUNK], F32R, tag="e",
                                        name="et")
                        nc.scalar.activation(out=et, in_=st, func=AF.Exp,
                                             scale=SCALE)
                        etiles.append(et)
                    for i in range(4):
                        mb = 4 * g + i
                        if probe == "half_o" and i % 2 == 1:
                            continue
                        nc.tensor.matmul(
                            oacc[c][0:65, :],
                            lhsT=vaugT[:, mb * 130 + h * 65:
                                       mb * 130 + h * 65 + 65],
                            rhs=etiles[i],
                            start=(mb == 0), stop=(mb == MB - 1))
        else:
            assert EXP_W == 2 * NCHUNK
            for mb in range(MB):
                i = mb % 4
                for c2 in range(NHALF // EXP_W):
                    st = psum.tile([128, EXP_W], F32, tag="s", bufs=SBUFS,
                                   name="st")
                    for hf in range(2):
                        cc = 2 * c2 + hf
                        nc.tensor.matmul(
                            st[:, hf * NCHUNK:(hf + 1) * NCHUNK],
                            lhsT=krep[32 * i:32 * (i + 1),
                                      mb * 128:(mb + 1) * 128],
                            rhs=qrep[32 * i:32 * (i + 1),
                                     cc * NCHUNK:(cc + 1) * NCHUNK],
                            start=True, stop=True, tile_position=(32 * i, 0))
                    et = epool.tile([128, EXP_W], F32R, tag="e", bufs=4,
                                    name="et")
                    nc.scalar.activation(out=et, in_=st, func=AF.Exp,
                                         scale=SCALE)
                    for hf in range(2):
                        cc = 2 * c2 + hf
                        nc.tensor.matmul(
                            oacc[cc][0:65, :],
                            lhsT=vaugT[:, mb * 130 + h * 65:
                                       mb * 130 + h * 65 + 65],
                            rhs=et[:, hf * NCHUNK:(hf + 1) * NCHUNK],
                            start=(mb == 0), stop=(mb == MB - 1))
        # normalize: rows 0:64 / row 64 (the ones-column accumulation)
        for c in range(NCH):
            rec = npool.tile([128, NCHUNK], F32R, tag="rec", name="rec")
            with nc.allow_low_precision(reason="f32r reciprocal feeds f32r matmul"):
                nc.vector.reciprocal(out=rec[64:65, :], in_=oacc[c][64:65, :])
            pb = psum.tile([128, NCHUNK], F32, tag="s", bufs=SBUFS, name="pb")
            nc.tensor.matmul(
                pb[0:64, :], lhsT=ones64[64:65, :],
                rhs=rec[64:65, :],
                start=True, stop=True, tile_position=(64, 0))
            rb = npool.tile([128, NCHUNK], F32, tag="rb", name="rb")
            nc.vector.tensor_copy(out=rb[0:64, :], in_=pb[0:64, :])
            nc.vector.tensor_mul(
                out=attn[h * 64:(h + 1) * 64, c * NCHUNK:(c + 1) * NCHUNK],
                in0=oacc[c][0:64, :], in1=rb[0:64, :])

    # ---- phase E: fused depthwise-conv + projection ----------------------
    psf = []
    for c in range(NCH):
        psf.append(psum.tile([128, NCHUNK], F32, tag="o", bufs=4,
                             name=f"psf{c}"))
    shifts = [(dy, dx) for dy in (-1, 0, 1) for dx in (-1, 0, 1)]
    for widx in range(10):
        for c in range(NCH):
            if widx == 0:
                lhsT = wblk(WPROJT)
                rhs = attn[:, c * NCHUNK:(c + 1) * NCHUNK]
            else:
                dy, dx = shifts[widx - 1]
                lhsT = wblk(MS0 + widx - 1)
                r0 = 1 + dy + 8 * c
                rhs = vpad3[:, r0:r0 + 8, 1 + dx:65 + dx]
            nc.tensor.matmul(psf[c], lhsT=lhsT,
                             rhs=rhs,
                             start=(widx == 0), stop=(widx == 9))
    for c in range(NCH):
        eng = copy_engines[c % 2]
        sl = slice(c * NCHUNK, (c + 1) * NCHUNK)
        if eng is nc.scalar:
            eng.copy(out=outsb[:, sl], in_=psf[c])
        else:
            eng.tensor_copy(out=outsb[:, sl], in_=psf[c])
        nc.sync.dma_start(out=io["out"][:, sl], in_=outsb[:, sl])

    for p in reversed(list(ctx_pools.values())):
        p.release()


def build_nc(reps=1):
    nc = bacc.Bacc(trn_type="TRN2", target_bir_lowering=False)
    io = {
        "wpack": nc.dram_tensor("wpack", [128, NWCOL], F32R,
                                kind="ExternalInput").ap(),
        "x_rot": nc.dram_tensor("x_rot", [128, N], F32R,
                                kind="ExternalInput").ap(),
        "x_q": nc.dram_tensor("x_q", [128, NHALF], F32R,
                              kind="ExternalInput").ap(),
        "out": nc.dram_tensor("out", [128, NHALF], F32,
                              kind="ExternalOutput").ap(),
    }
    with tile.TileContext(nc) as tc:
        if reps == 1:
            _emit_kernel(tc, io)
        else:
            with tc.For_i(0, reps, 1):
                _emit_kernel(tc, io)
    nc.compile()
    return nc


def host_prep(x, w_qkv, w_pe, w_proj):
    """Build the 8 per-core input maps from the full problem inputs."""
    x = np.ascontiguousarray(x, dtype=np.float32)
    wq = np.asarray(w_qkv, dtype=np.float32)[:, :, 0, 0]      # [256,128]
    wpe = np.asarray(w_pe, dtype=np.float32)[:, 0]            # [128,3,3]
    wpj = np.asarray(w_proj, dtype=np.float32)[:, :, 0, 0]    # [128,128]

    blocks = []
    for h in range(2):
        blocks.append(np.tile(wq[h * 128:h * 128 + 32], (4, 1)).T)       # WQh
    for h in range(2):
        blocks.append(np.tile(wq[h * 128 + 32:h * 128 + 64], (4, 1)).T)  # WKh
    blocks.insert(4, np.concatenate(
        [wq[64:128], wq[192:256]], axis=0).T)                 # WV01
    blocks.append(wpj.T)                                      # WPROJT
    for dy in (-1, 0, 1):
        for dx in (-1, 0, 1):
            blocks.append((wpj * wpe[:, dy + 1, dx + 1][None, :]).T)
    blocks.append(np.eye(128, dtype=np.float32))              # IDENT
    wpack_base = np.concatenate(blocks, axis=1)               # [128, 16*128]

    in_maps = []
    for core in range(8):
        b, half = core // 2, core % 2
        y0 = 32 * half
        halo = np.zeros((128, 4), np.float32)
        halo[:, 0] = 1.0 if half == 1 else 0.0    # top halo valid?
        halo[:, 1] = 1.0 if half == 0 else 0.0    # bottom halo valid?
        halo[:, 2] = 1.0                          # const one
        halo[:, 3] = 0.0                          # const zero
        wpack = np.concatenate([wpack_base, halo], axis=1)
        x_rot = np.roll(x[b], 1 - y0, axis=1).reshape(128, N)
        x_q = x[b][:, y0:y0 + 32, :].reshape(128, NHALF)
        in_maps.append({
            "wpack": np.ascontiguousarray(wpack),
            "x_rot": np.ascontiguousarray(x_rot),
            "x_q": np.ascontiguousarray(x_q),
        })
    return in_maps


def assemble(results):
    out = np.zeros((B, C, H, W), np.float32)
    for core in range(8):
        b, half = core // 2, core % 2
        out[b, :, 32 * half:32 * half + 32, :] = \
            results[core]["out"].reshape(C, 32, W)
    return out


_NC_CACHE = {}


def _get_nc(reps=1):
    if reps not in _NC_CACHE:
        _NC_CACHE[reps] = build_nc(reps)
    return _NC_CACHE[reps]


def run(x, w_qkv, w_pe, w_proj, reps=1, **spmd_kwargs):
    nc = _get_nc(reps)
    in_maps = host_prep(x, w_qkv, w_pe, w_proj)
    res = run_bass_kernel_spmd(nc, in_maps, core_ids=list(range(8)),
                               **spmd_kwargs)
    return assemble(res.results), res


def kernel(x, w_qkv, w_pe, w_proj):
    out, _ = run(x, w_qkv, w_pe, w_proj)
    return out


# revision 24
# speedup vs baseline: 1.8295x; 1.8295x over previous
"""Trainium2 Bass kernel for nn_Attention_11192684774105.

Reference computation (B=4, C=128, H=W=64, N=4096, 2 heads, key_dim=32,
head_dim=64):
    qkv  = conv1x1(x, w_qkv)                    # [B,256,H,W]
    q,k,v split per head; attn = softmax(q^T k / sqrt(32)) over keys
    out  = v @ attn^T  (+ depthwise3x3(v, w_pe)) -> conv1x1(w_proj)

Sharding: 8 cores = (batch b, row-half) pairs.  Each core computes both
heads for 2048 query positions (32 image rows) of one sample; keys/values
span the full 4096 positions.  Output is a pure concatenation.

Per-core algorithm (all matmuls on PE in float32r):
  - K/V are computed from a row-rotated copy of x so the depthwise-conv
    halo rows sit at fixed positions (softmax/attention are invariant to
    a consistent permutation of the key axis).
  - S^T[m,n] = K^T Q is computed with m (key) on partitions, n in free dim,
    exp() runs on the scalar engine (no max subtraction needed: |logits|
    < 3), and the softmax denominator comes from an extra ones-column in
    the V^T stationary operand of the second matmul (row 64 of the PSUM
    accumulator = sum_m exp).
  - The depthwise conv + output projection are fused into 10 accumulating
    1x1 matmuls: W_proj @ attn_out + sum_s (W_proj * w_pe[:,s]) @ shift_s(V).
"""

import os

import numpy as np

import concourse.bass as bass
import concourse.mybir as mybir
import concourse.tile as tile
from concourse import bacc
from concourse.bass_utils import run_bass_kernel_spmd

F32 = mybir.dt.float32
F32R = mybir.dt.float32r
BF16 = mybir.dt.bfloat16
AF = mybir.ActivationFunctionType

B, C, H, W = 4, 128, 64, 64
N = H * W                    # 4096
NHALF = N // 2               # 2048 query positions per core
SCALE = 32 ** (-0.5)
NCHUNK = 512                 # matmul moving free dim / psum bank
NCH = NHALF // NCHUNK        # 4 chunks per core
MB = N // 128                # 32 key blocks of 128
EXP_W = 512                  # free width of each scalar-engine exp op
SBUFS = 4 if EXP_W == NCHUNK else 2   # S-psum slots (x EXP_W//512 banks)

# wpack column-block indices (each block is 128 cols)
WQ0, WQ1, WK0, WK1, WV01, WPROJT, MS0, IDENT = 0, 1, 2, 3, 4, 5, 6, 15
NWCOL = 16 * 128 + 4   # + halo_top, halo_bot, const-1.0, const-0.0 cols


def _emit_kernel(tc, io):
    """Emit the per-core program. io: dict of DRAM APs."""
    nc = tc.nc
    core_bf16 = os.environ.get("KCORE", "bf16") == "bf16"
    CDT = BF16 if core_bf16 else F32R
    no_dma = os.environ.get("KPROBE", "") == "no_dma"
    ctx_pools = {}

    const = tc.alloc_tile_pool(name="const", bufs=1)
    epool = tc.alloc_tile_pool(name="epool", bufs=8)
    npool = tc.alloc_tile_pool(name="npool", bufs=2)
    psum = tc.alloc_tile_pool(name="psum", bufs=1, space="PSUM")
    ctx_pools.update(const=const, epool=epool, npool=npool, psum=psum)

    # ---- load inputs (chunked so consumers start early) ------------------
    wp = const.tile([128, NWCOL], F32R)
    xr = const.tile([128, N], F32R)
    xq = const.tile([128, NHALF], F32R)
    if not no_dma:
        # qkv weight blocks first, then identity/halo/consts, proj last
        nc.sync.dma_start(out=wp[:, 0:5 * 128], in_=io["wpack"][:, 0:5 * 128])
        nc.sync.dma_start(out=wp[:, 15 * 128:NWCOL],
                          in_=io["wpack"][:, 15 * 128:NWCOL])
        nc.sync.dma_start(out=wp[:, 5 * 128:15 * 128],
                          in_=io["wpack"][:, 5 * 128:15 * 128])
        for j in range(4):
            nc.sync.dma_start(out=xr[:, j * 1024:(j + 1) * 1024],
                              in_=io["x_rot"][:, j * 1024:(j + 1) * 1024])
        for j in range(2):
            nc.sync.dma_start(out=xq[:, j * 1024:(j + 1) * 1024],
                              in_=io["x_q"][:, j * 1024:(j + 1) * 1024])

    # ---- persistent sbuf -------------------------------------------------
    krep0 = const.tile([128, N], CDT)
    krep1 = const.tile([128, N], CDT)
    qrep0 = const.tile([128, NHALF], CDT)
    qrep1 = const.tile([128, NHALF], CDT)
    vchan = const.tile([128, N], CDT)
    vaugT = const.tile([128, MB * 130], CDT)
    vpad = const.tile([128, 34 * 66], F32R)
    attn = const.tile([128, NHALF], F32R)
    outsb = const.tile([128, NHALF], F32)
    ones64 = const.tile([128, 64], F32R)

    one_col = wp[:, 16 * 128 + 2:16 * 128 + 3]
    zero_col = wp[:, 16 * 128 + 3:16 * 128 + 4]
    # Pin the exp table set before any ACT Copy picks a different one.
    actwarm = npool.tile([128, 1], F32, tag="actwarm", name="actwarm")
    nc.scalar.activation(out=actwarm, in_=one_col, func=AF.Exp, scale=1.0)
    # Only the aug-ones columns (64,129 mod 130) and vpad border columns
    # (0,65 mod 66) actually need filling; everything else is overwritten.
    vaug4 = vaugT.rearrange("p (mb c) -> p mb c", mb=MB, c=130)
    nc.vector.tensor_copy(out=vaug4[:, :, 64:130:65],
                          in_=one_col.broadcast_to([128, MB, 2]))
    vpadr = vpad.rearrange("p (r c) -> p r c", r=34, c=66)
    nc.vector.tensor_copy(out=vpadr[:, :, 0:66:65],
                          in_=zero_col.broadcast_to([128, 34, 2]))
    nc.vector.tensor_copy(out=ones64, in_=one_col.broadcast_to([128, 64]))

    def wblk(i):
        return wp[:, i * 128:(i + 1) * 128]

    copy_engines = [nc.vector, nc.scalar]

    # ---- phase A: qkv 1x1 conv (weights pre-transposed/replicated) -------
    qkv_specs = [
        (WK0, xr, krep0, N), (WQ0, xq, qrep0, NHALF), (WV01, xr, vchan, N),
        (WK1, xr, krep1, N), (WQ1, xq, qrep1, NHALF),
    ]
    ci = 0
    for blk, src, dst, width in qkv_specs:
        for j in range(width // NCHUNK):
            ps = psum.tile([128, NCHUNK], F32, tag="s", bufs=6,
                           name="ps_a")
            nc.tensor.matmul(
                ps, lhsT=wblk(blk),
                rhs=src[:, j * NCHUNK:(j + 1) * NCHUNK],
                start=True, stop=True)
            eng = copy_engines[ci % 2]
            ci += 1
            if eng is nc.scalar:
                eng.copy(out=dst[:, j * NCHUNK:(j + 1) * NCHUNK], in_=ps)
            else:
                eng.tensor_copy(out=dst[:, j * NCHUNK:(j + 1) * NCHUNK], in_=ps)

    # ---- phase B: V^T blocks into augmented stationary layout ------------
    # vaugT block mb (130 cols): [v0^T(64) | 1 | v1^T(64) | 1]
    if core_bf16:
        identity = const.tile([128, 128], BF16)
        nc.gpsimd.memset(identity, 0.0)
        nc.gpsimd.affine_select(
            out=identity, in_=identity,
            compare_op=mybir.AluOpType.not_equal, fill=1.0, base=0,
            pattern=[[-1, 128]], channel_multiplier=1)
    else:
        identity = wblk(IDENT)
    for mb in range(MB):
        pt = psum.tile([128, 128], CDT, tag="s", bufs=6, name="pt")
        nc.tensor.transpose(pt, vchan[:, mb * 128:(mb + 1) * 128], identity)
        dst = vaugT[:, mb * 130:mb * 130 + 130].rearrange(
            "p (a b) -> p a b", a=2, b=65)[:, :, 0:64]
        src = pt.rearrange("p (a b) -> p a b", a=2, b=64)
        nc.vector.tensor_copy(out=dst, in_=src)

    # ---- phase D: zero-padded V for the depthwise conv -------------------
    vpad3 = vpad.rearrange("p (r c) -> p r c", r=34, c=66)
    nc.vector.tensor_copy(out=vpad3[:, 1:33, 1:65], in_=vchan[:, 64:64 + 32 * 64])
    nc.vector.tensor_scalar(
        out=vpad3[:, 0, 1:65], in0=vchan[:, 0:64],
        scalar1=wp[:, 16 * 128:16 * 128 + 1].bitcast(F32), scalar2=None,
        op0=mybir.AluOpType.mult)
    nc.vector.tensor_scalar(
        out=vpad3[:, 33, 1:65], in0=vchan[:, 33 * 64:34 * 64],
        scalar1=wp[:, 16 * 128 + 1:16 * 128 + 2].bitcast(F32), scalar2=None,
        op0=mybir.AluOpType.mult)

    # ---- phase C: attention per head ------------------------------------
    # Two passes over the query chunks per head: the O accumulator then
    # needs only 2 PSUM banks, freeing 6 banks for a deep S^T pipeline so
    # the PE never stalls at its strict-FIFO queue head.
    probe = os.environ.get("KPROBE", "")
    LAG = 3
    UMB = 2                       # key-blocks per unit
    for h in range(2):
        krep = (krep0, krep1)[h]
        qrep = (qrep0, qrep1)[h]
        for npass in range(2):
            oacc = {}
            for cl in range(2):
                c = 2 * npass + cl
                oacc[c] = psum.tile([128, NCHUNK], F32, tag="o", bufs=2,
                                    name=f"oacc{cl}")
            pend = []

            lastmb = MB - 2 if probe == "half_o" else MB - 1

            def flush_o(item):
                c, lst = item
                for mb, et in lst:
                    if probe == "half_o" and mb % 2 == 1:
                        continue
                    nc.tensor.matmul(
                        oacc[c][0:65, :],
                        lhsT=vaugT[:, mb * 130 + h * 65:
                                   mb * 130 + h * 65 + 65],
                        rhs=et,
                        start=(mb == 0), stop=(mb == lastmb))

            for g in range(MB // UMB):
                for cl in range(2):
                    c = 2 * npass + cl
                    lst = []
                    for i2 in range(UMB):
                        mb = UMB * g + i2
                        rg = 0 if probe == "no_tilepos" else mb % 4
                        if probe == "half_st" and i2 == 1:
                            lst.append((mb, lst[-1][1]))
                            continue
                        st = psum.tile([128, NCHUNK], F32, tag="s",
                                       bufs=6, name="st")
                        nc.tensor.matmul(
                            st,
                            lhsT=krep[32 * rg:32 * (rg + 1),
                                      mb * 128:(mb + 1) * 128],
                            rhs=qrep[32 * rg:32 * (rg + 1),
                                     c * NCHUNK:(c + 1) * NCHUNK],
                            start=True, stop=True,
                            tile_position=(32 * rg, 0))
                        if probe == "half_exp" and i2 == 1:
                            lst.append((mb, lst[-1][1]))
                            continue
                        et = epool.tile([128, NCHUNK], CDT, tag="e",
                                        name="et")
                        nc.scalar.activation(out=et, in_=st, func=AF.Exp,
                                             scale=SCALE)
                        lst.append((mb, et))
                    pend.append((c, lst))
                    if len(pend) > LAG:
                        flush_o(pend.pop(0))
            for item in pend:
                flush_o(item)
            # normalize: rows 0:64 / row 64 (the ones-column accumulation)
            for cl in range(2):
                c = 2 * npass + cl
                rec = npool.tile([128, NCHUNK], F32R, tag="rec", name="rec")
                with nc.allow_low_precision(reason="f32r recip for f32r mm"):
                    nc.vector.reciprocal(out=rec[64:65, :],
                                         in_=oacc[c][64:65, :])
                pb = psum.tile([128, NCHUNK], F32, tag="s", bufs=6,
                               name="pb")
                nc.tensor.matmul(
                    pb[0:64, :], lhsT=ones64[64:65, :],
                    rhs=rec[64:65, :],
                    start=True, stop=True, tile_position=(64, 0))
                rb = npool.tile([128, NCHUNK], F32, tag="rb", name="rb")
                nc.vector.tensor_copy(out=rb[0:64, :], in_=pb[0:64, :])
                nc.vector.tensor_mul(
                    out=attn[h * 64:(h + 1) * 64,
                             c * NCHUNK:(c + 1) * NCHUNK],
                    in0=oacc[c][0:64, :], in1=rb[0:64, :])

    # ---- phase E: fused depthwise-conv + projection ----------------------
    psf = []
    for c in range(NCH):
        psf.append(psum.tile([128, NCHUNK], F32, tag="s", bufs=6,
                             name=f"psf{c}"))
    shifts = [(dy, dx) for dy in (-1, 0, 1) for dx in (-1, 0, 1)]
    for widx in range(10):
        for c in range(NCH):
            if widx == 0:
                lhsT = wblk(WPROJT)
                rhs = attn[:, c * NCHUNK:(c + 1) * NCHUNK]
            else:
                dy, dx = shifts[widx - 1]
                lhsT = wblk(MS0 + widx - 1)
                r0 = 1 + dy + 8 * c
                rhs = vpad3[:, r0:r0 + 8, 1 + dx:65 + dx]
            nc.tensor.matmul(psf[c], lhsT=lhsT,
                             rhs=rhs,
                             start=(widx == 0), stop=(widx == 9))
    for c in range(NCH):
        eng = copy_engines[c % 2]
        sl = slice(c * NCHUNK, (c + 1) * NCHUNK)
        if eng is nc.scalar:
            eng.copy(out=outsb[:, sl], in_=psf[c])
        else:
            eng.tensor_copy(out=outsb[:, sl], in_=psf[c])
        nc.sync.dma_start(out=io["out"][:, sl], in_=outsb[:, sl])

    for p in reversed(list(ctx_pools.values())):
        p.release()


def build_nc(reps=1):
    nc = bacc.Bacc(trn_type="TRN2", target_bir_lowering=False)
    io = {
        "wpack": nc.dram_tensor("wpack", [128, NWCOL], F32R,
                                kind="ExternalInput").ap(),
        "x_rot": nc.dram_tensor("x_rot", [128, N], F32R,
                                kind="ExternalInput").ap(),
        "x_q": nc.dram_tensor("x_q", [128, NHALF], F32R,
                              kind="ExternalInput").ap(),
        "out": nc.dram_tensor("out", [128, NHALF], F32,
                              kind="ExternalOutput").ap(),
    }
    with tile.TileContext(nc) as tc:
        if reps == 1:
            _emit_kernel(tc, io)
        else:
            with tc.For_i(0, reps, 1):
                _emit_kernel(tc, io)
    nc.compile()
    return nc


def host_prep(x, w_qkv, w_pe, w_proj):
    """Build the 8 per-core input maps from the full problem inputs."""
    x = np.ascontiguousarray(x, dtype=np.float32)
    wq = np.asarray(w_qkv, dtype=np.float32)[:, :, 0, 0]      # [256,128]
    wpe = np.asarray(w_pe, dtype=np.float32)[:, 0]            # [128,3,3]
    wpj = np.asarray(w_proj, dtype=np.float32)[:, :, 0, 0]    # [128,128]

    blocks = []
    for h in range(2):
        blocks.append(np.tile(wq[h * 128:h * 128 + 32], (4, 1)).T)       # WQh
    for h in range(2):
        blocks.append(np.tile(wq[h * 128 + 32:h * 128 + 64], (4, 1)).T)  # WKh
    blocks.insert(4, np.concatenate(
        [wq[64:128], wq[192:256]], axis=0).T)                 # WV01
    blocks.append(wpj.T)                                      # WPROJT
    for dy in (-1, 0, 1):
        for dx in (-1, 0, 1):
            blocks.append((wpj * wpe[:, dy + 1, dx + 1][None, :]).T)
    blocks.append(np.eye(128, dtype=np.float32))              # IDENT
    wpack_base = np.concatenate(blocks, axis=1)               # [128, 16*128]

    in_maps = []
    for core in range(8):
        b, half = core // 2, core % 2
        y0 = 32 * half
        halo = np.zeros((128, 4), np.float32)
        halo[:, 0] = 1.0 if half == 1 else 0.0    # top halo valid?
        halo[:, 1] = 1.0 if half == 0 else 0.0    # bottom halo valid?
        halo[:, 2] = 1.0                          # const one
        halo[:, 3] = 0.0                          # const zero
        wpack = np.concatenate([wpack_base, halo], axis=1)
        x_rot = np.roll(x[b], 1 - y0, axis=1).reshape(128, N)
        x_q = x[b][:, y0:y0 + 32, :].reshape(128, NHALF)
        in_maps.append({
            "wpack": np.ascontiguousarray(wpack),
            "x_rot": np.ascontiguousarray(x_rot),
            "x_q": np.ascontiguousarray(x_q),
        })
    return in_maps


def assemble(results):
    out = np.zeros((B, C, H, W), np.float32)
    for core in range(8):
        b, half = core // 2, core % 2
        out[b, :, 32 * half:32 * half + 32, :] = \
            results[core]["out"].reshape(C, 32, W)
    return out


_NC_CACHE = {}


def _get_nc(reps=1):
    if reps not in _NC_CACHE:
        _NC_CACHE[reps] = build_nc(reps)
    return _NC_CACHE[reps]


def run(x, w_qkv, w_pe, w_proj, reps=1, **spmd_kwargs):
    nc = _get_nc(reps)
    in_maps = host_prep(x, w_qkv, w_pe, w_proj)
    res = run_bass_kernel_spmd(nc, in_maps, core_ids=list(range(8)),
                               **spmd_kwargs)
    return assemble(res.results), res


def kernel(x, w_qkv, w_pe, w_proj):
    out, _ = run(x, w_qkv, w_pe, w_proj)
    return out


# revision 26
# speedup vs baseline: 2.4505x; 1.3394x over previous
"""Trainium2 Bass kernel for nn_Attention_11192684774105.

Reference computation (B=4, C=128, H=W=64, N=4096, 2 heads, key_dim=32,
head_dim=64):
    qkv  = conv1x1(x, w_qkv)                    # [B,256,H,W]
    q,k,v split per head; attn = softmax(q^T k / sqrt(32)) over keys
    out  = v @ attn^T  (+ depthwise3x3(v, w_pe)) -> conv1x1(w_proj)

Sharding: 8 cores = (batch b, row-half) pairs.  Each core computes both
heads for 2048 query positions (32 image rows) of one sample; keys/values
span the full 4096 positions.  Output is a pure concatenation.

Per-core algorithm (all matmuls on PE in float32r):
  - K/V are computed from a row-rotated copy of x so the depthwise-conv
    halo rows sit at fixed positions (softmax/attention are invariant to
    a consistent permutation of the key axis).
  - S^T[m,n] = K^T Q is computed with m (key) on partitions, n in free dim,
    exp() runs on the scalar engine (no max subtraction needed: |logits|
    < 3), and the softmax denominator comes from an extra ones-column in
    the V^T stationary operand of the second matmul (row 64 of the PSUM
    accumulator = sum_m exp).
  - The depthwise conv + output projection are fused into 10 accumulating
    1x1 matmuls: W_proj @ attn_out + sum_s (W_proj * w_pe[:,s]) @ shift_s(V).
"""

import os

import numpy as np

import concourse.bass as bass
import concourse.mybir as mybir
import concourse.tile as tile
from concourse import bacc
from concourse.bass_utils import run_bass_kernel_spmd

F32 = mybir.dt.float32
F32R = mybir.dt.float32r
BF16 = mybir.dt.bfloat16
AF = mybir.ActivationFunctionType

B, C, H, W = 4, 128, 64, 64
N = H * W                    # 4096
NHALF = N // 2               # 2048 query positions per core
SCALE = 32 ** (-0.5)
NCHUNK = 512                 # matmul moving free dim / psum bank
NCH = NHALF // NCHUNK        # 4 chunks per core
MB = N // 128                # 32 key blocks of 128
EXP_W = 512                  # free width of each scalar-engine exp op
SBUFS = 4 if EXP_W == NCHUNK else 2   # S-psum slots (x EXP_W//512 banks)

# wpack column-block indices (each block is 128 cols)
WQ0, WQ1, WK0, WK1, WV01, WPROJT, MS0, IDENT = 0, 1, 2, 3, 4, 5, 6, 15
NWCOL = 16 * 128 + 4   # + halo_top, halo_bot, const-1.0, const-0.0 cols


def _emit_kernel(tc, io):
    """Emit the per-core program. io: dict of DRAM APs."""
    nc = tc.nc
    core_bf16 = os.environ.get("KCORE", "bf16") == "bf16"
    CDT = BF16 if core_bf16 else F32R
    no_dma = os.environ.get("KPROBE", "") == "no_dma"
    ctx_pools = {}

    const = tc.alloc_tile_pool(name="const", bufs=1)
    epool = tc.alloc_tile_pool(name="epool", bufs=8)
    npool = tc.alloc_tile_pool(name="npool", bufs=2)
    psum = tc.alloc_tile_pool(name="psum", bufs=1, space="PSUM")
    ctx_pools.update(const=const, epool=epool, npool=npool, psum=psum)

    # ---- load inputs (chunked so consumers start early) ------------------
    wp = const.tile([128, NWCOL], F32R)
    xr = const.tile([128, N], F32R)
    xq = const.tile([128, NHALF], F32R)
    if not no_dma:
        # qkv weight blocks first, then identity/halo/consts, proj last
        nc.sync.dma_start(out=wp[:, 0:5 * 128], in_=io["wpack"][:, 0:5 * 128])
        nc.sync.dma_start(out=wp[:, 15 * 128:NWCOL],
                          in_=io["wpack"][:, 15 * 128:NWCOL])
        nc.sync.dma_start(out=wp[:, 5 * 128:15 * 128],
                          in_=io["wpack"][:, 5 * 128:15 * 128])
        for j in range(4):
            nc.sync.dma_start(out=xr[:, j * 1024:(j + 1) * 1024],
                              in_=io["x_rot"][:, j * 1024:(j + 1) * 1024])
        for j in range(2):
            nc.sync.dma_start(out=xq[:, j * 1024:(j + 1) * 1024],
                              in_=io["x_q"][:, j * 1024:(j + 1) * 1024])

    # ---- persistent sbuf -------------------------------------------------
    krep0 = const.tile([128, N], CDT)
    krep1 = const.tile([128, N], CDT)
    qrep0 = const.tile([128, NHALF], CDT)
    qrep1 = const.tile([128, NHALF], CDT)
    vchan = const.tile([128, N], CDT)
    vaugT = const.tile([128, MB * 130], CDT)
    vpad = const.tile([128, 34 * 66], F32R)
    attn = const.tile([128, NHALF], F32R)
    outsb = const.tile([128, NHALF], F32)
    ones64 = const.tile([128, 64], F32R)

    one_col = wp[:, 16 * 128 + 2:16 * 128 + 3]
    zero_col = wp[:, 16 * 128 + 3:16 * 128 + 4]
    # Pin the exp table set before any ACT Copy picks a different one.
    actwarm = npool.tile([128, 1], F32, tag="actwarm", name="actwarm")
    nc.scalar.activation(out=actwarm, in_=one_col, func=AF.Exp, scale=1.0)
    # Only the aug-ones columns (64,129 mod 130) and vpad border columns
    # (0,65 mod 66) actually need filling; everything else is overwritten.
    vaug4 = vaugT.rearrange("p (mb c) -> p mb c", mb=MB, c=130)
    nc.vector.tensor_copy(out=vaug4[:, :, 64:130:65],
                          in_=one_col.broadcast_to([128, MB, 2]))
    vpadr = vpad.rearrange("p (r c) -> p r c", r=34, c=66)
    nc.vector.tensor_copy(out=vpadr[:, :, 0:66:65],
                          in_=zero_col.broadcast_to([128, 34, 2]))
    nc.vector.tensor_copy(out=ones64, in_=one_col.broadcast_to([128, 64]))

    def wblk(i):
        return wp[:, i * 128:(i + 1) * 128]

    copy_engines = [nc.vector, nc.scalar]

    # ---- phase A: qkv 1x1 conv (weights pre-transposed/replicated) -------
    qkv_specs = [
        (WK0, xr, krep0, N), (WQ0, xq, qrep0, NHALF), (WV01, xr, vchan, N),
        (WK1, xr, krep1, N), (WQ1, xq, qrep1, NHALF),
    ]
    ci = 0
    for blk, src, dst, width in qkv_specs:
        for j in range(width // NCHUNK):
            ps = psum.tile([128, NCHUNK], F32, tag="s", bufs=3,
                           name="ps_a")
            nc.tensor.matmul(
                ps, lhsT=wblk(blk),
                rhs=src[:, j * NCHUNK:(j + 1) * NCHUNK],
                start=True, stop=True)
            eng = copy_engines[ci % 2]
            ci += 1
            if eng is nc.scalar:
                eng.copy(out=dst[:, j * NCHUNK:(j + 1) * NCHUNK], in_=ps)
            else:
                eng.tensor_copy(out=dst[:, j * NCHUNK:(j + 1) * NCHUNK], in_=ps)

    # ---- phase B: V^T blocks into augmented stationary layout ------------
    # vaugT block mb (130 cols): [v0^T(64) | 1 | v1^T(64) | 1]
    if core_bf16:
        identity = const.tile([128, 128], BF16)
        nc.gpsimd.memset(identity, 0.0)
        nc.gpsimd.affine_select(
            out=identity, in_=identity,
            compare_op=mybir.AluOpType.not_equal, fill=1.0, base=0,
            pattern=[[-1, 128]], channel_multiplier=1)
    else:
        identity = wblk(IDENT)
    for mb in range(MB):
        pt = psum.tile([128, 128], CDT, tag="s", bufs=3, name="pt")
        nc.tensor.transpose(pt, vchan[:, mb * 128:(mb + 1) * 128], identity)
        dst = vaugT[:, mb * 130:mb * 130 + 130].rearrange(
            "p (a b) -> p a b", a=2, b=65)[:, :, 0:64]
        src = pt.rearrange("p (a b) -> p a b", a=2, b=64)
        nc.vector.tensor_copy(out=dst, in_=src)

    # ---- phase D: zero-padded V for the depthwise conv -------------------
    vpad3 = vpad.rearrange("p (r c) -> p r c", r=34, c=66)
    nc.vector.tensor_copy(out=vpad3[:, 1:33, 1:65], in_=vchan[:, 64:64 + 32 * 64])
    nc.vector.tensor_scalar(
        out=vpad3[:, 0, 1:65], in0=vchan[:, 0:64],
        scalar1=wp[:, 16 * 128:16 * 128 + 1].bitcast(F32), scalar2=None,
        op0=mybir.AluOpType.mult)
    nc.vector.tensor_scalar(
        out=vpad3[:, 33, 1:65], in0=vchan[:, 33 * 64:34 * 64],
        scalar1=wp[:, 16 * 128 + 1:16 * 128 + 2].bitcast(F32), scalar2=None,
        op0=mybir.AluOpType.mult)

    # ---- phase C: attention per head ------------------------------------
    # Two passes over the query chunks per head: the O accumulator then
    # needs only 2 PSUM banks, freeing 6 banks for a deep S^T pipeline so
    # the PE never stalls at its strict-FIFO queue head.
    probe = os.environ.get("KPROBE", "")
    LAG = 3
    UMB = 2                       # key-blocks per unit
    for h in range(2):
        krep = (krep0, krep1)[h]
        qrep = (qrep0, qrep1)[h]
        for npass in range(2):
            oacc = {}
            for cl in range(2):
                c = 2 * npass + cl
                oacc[c] = psum.tile([128, NCHUNK], F32, tag="o", bufs=2,
                                    name=f"oacc{cl}")
            pend = []

            def flush_o(c, mb, et):
                nc.tensor.matmul(
                    oacc[c][0:65, :],
                    lhsT=vaugT[:, mb * 130 + h * 65:
                               mb * 130 + h * 65 + 65],
                    rhs=et,
                    start=(mb == 0), stop=(mb == MB - 1))

            for mb in range(MB):
                rg = 0 if probe == "no_tilepos" else mb % 4
                c0 = 2 * npass
                st = psum.tile([128, 2 * NCHUNK], F32, tag="s",
                               bufs=3, name="st")
                for cl in range(2):
                    nc.tensor.matmul(
                        st[:, cl * NCHUNK:(cl + 1) * NCHUNK],
                        lhsT=krep[32 * rg:32 * (rg + 1),
                                  mb * 128:(mb + 1) * 128],
                        rhs=qrep[32 * rg:32 * (rg + 1),
                                 (c0 + cl) * NCHUNK:(c0 + cl + 1) * NCHUNK],
                        start=True, stop=True,
                        tile_position=(32 * rg, 0))
                et = epool.tile([128, 2 * NCHUNK], CDT, tag="e", name="et")
                nc.scalar.activation(out=et, in_=st, func=AF.Exp,
                                     scale=SCALE)
                pend.append((mb, et))
                if len(pend) > LAG:
                    mb_o, et_o = pend.pop(0)
                    for cl in range(2):
                        flush_o(c0 + cl, mb_o,
                                et_o[:, cl * NCHUNK:(cl + 1) * NCHUNK])
            for mb_o, et_o in pend:
                for cl in range(2):
                    flush_o(2 * npass + cl, mb_o,
                            et_o[:, cl * NCHUNK:(cl + 1) * NCHUNK])
            # normalize: rows 0:64 / row 64 (the ones-column accumulation)
            for cl in range(2):
                c = 2 * npass + cl
                rec = npool.tile([128, NCHUNK], F32R, tag="rec", name="rec")
                with nc.allow_low_precision(reason="f32r recip for f32r mm"):
                    nc.vector.reciprocal(out=rec[64:65, :],
                                         in_=oacc[c][64:65, :])
                pb = psum.tile([128, NCHUNK], F32, tag="s", bufs=3,
                               name="pb")
                nc.tensor.matmul(
                    pb[0:64, :], lhsT=ones64[64:65, :],
                    rhs=rec[64:65, :],
                    start=True, stop=True, tile_position=(64, 0))
                rb = npool.tile([128, NCHUNK], F32, tag="rb", name="rb")
                nc.vector.tensor_copy(out=rb[0:64, :], in_=pb[0:64, :])
                nc.vector.tensor_mul(
                    out=attn[h * 64:(h + 1) * 64,
                             c * NCHUNK:(c + 1) * NCHUNK],
                    in0=oacc[c][0:64, :], in1=rb[0:64, :])

    # ---- phase E: fused depthwise-conv + projection ----------------------
    psf = []
    for c in range(NCH):
        psf.append(psum.tile([128, NCHUNK], F32, tag="s", bufs=3,
                             name=f"psf{c}"))
    shifts = [(dy, dx) for dy in (-1, 0, 1) for dx in (-1, 0, 1)]
    for widx in range(10):
        for c in range(NCH):
            if widx == 0:
                lhsT = wblk(WPROJT)
                rhs = attn[:, c * NCHUNK:(c + 1) * NCHUNK]
            else:
                dy, dx = shifts[widx - 1]
                lhsT = wblk(MS0 + widx - 1)
                r0 = 1 + dy + 8 * c
                rhs = vpad3[:, r0:r0 + 8, 1 + dx:65 + dx]
            nc.tensor.matmul(psf[c], lhsT=lhsT,
                             rhs=rhs,
                             start=(widx == 0), stop=(widx == 9))
    for c in range(NCH):
        eng = copy_engines[c % 2]
        sl = slice(c * NCHUNK, (c + 1) * NCHUNK)
        if eng is nc.scalar:
            eng.copy(out=outsb[:, sl], in_=psf[c])
        else:
            eng.tensor_copy(out=outsb[:, sl], in_=psf[c])
        nc.sync.dma_start(out=io["out"][:, sl], in_=outsb[:, sl])

    for p in reversed(list(ctx_pools.values())):
        p.release()


def build_nc(reps=1):
    nc = bacc.Bacc(trn_type="TRN2", target_bir_lowering=False)
    io = {
        "wpack": nc.dram_tensor("wpack", [128, NWCOL], F32R,
                                kind="ExternalInput").ap(),
        "x_rot": nc.dram_tensor("x_rot", [128, N], F32R,
                                kind="ExternalInput").ap(),
        "x_q": nc.dram_tensor("x_q", [128, NHALF], F32R,
                              kind="ExternalInput").ap(),
        "out": nc.dram_tensor("out", [128, NHALF], F32,
                              kind="ExternalOutput").ap(),
    }
    with tile.TileContext(nc) as tc:
        if reps == 1:
            _emit_kernel(tc, io)
        else:
            with tc.For_i(0, reps, 1):
                _emit_kernel(tc, io)
    nc.compile()
    return nc


def host_prep(x, w_qkv, w_pe, w_proj):
    """Build the 8 per-core input maps from the full problem inputs."""
    x = np.ascontiguousarray(x, dtype=np.float32)
    wq = np.asarray(w_qkv, dtype=np.float32)[:, :, 0, 0]      # [256,128]
    wpe = np.asarray(w_pe, dtype=np.float32)[:, 0]            # [128,3,3]
    wpj = np.asarray(w_proj, dtype=np.float32)[:, :, 0, 0]    # [128,128]

    blocks = []
    for h in range(2):
        blocks.append(np.tile(wq[h * 128:h * 128 + 32], (4, 1)).T)       # WQh
    for h in range(2):
        blocks.append(np.tile(wq[h * 128 + 32:h * 128 + 64], (4, 1)).T)  # WKh
    blocks.insert(4, np.concatenate(
        [wq[64:128], wq[192:256]], axis=0).T)                 # WV01
    blocks.append(wpj.T)                                      # WPROJT
    for dy in (-1, 0, 1):
        for dx in (-1, 0, 1):
            blocks.append((wpj * wpe[:, dy + 1, dx + 1][None, :]).T)
    blocks.append(np.eye(128, dtype=np.float32))              # IDENT
    wpack_base = np.concatenate(blocks, axis=1)               # [128, 16*128]

    in_maps = []
    for core in range(8):
        b, half = core // 2, core % 2
        y0 = 32 * half
        halo = np.zeros((128, 4), np.float32)
        halo[:, 0] = 1.0 if half == 1 else 0.0    # top halo valid?
        halo[:, 1] = 1.0 if half == 0 else 0.0    # bottom halo valid?
        halo[:, 2] = 1.0                          # const one
        halo[:, 3] = 0.0                          # const zero
        wpack = np.concatenate([wpack_base, halo], axis=1)
        x_rot = np.roll(x[b], 1 - y0, axis=1).reshape(128, N)
        x_q = x[b][:, y0:y0 + 32, :].reshape(128, NHALF)
        in_maps.append({
            "wpack": np.ascontiguousarray(wpack),
            "x_rot": np.ascontiguousarray(x_rot),
            "x_q": np.ascontiguousarray(x_q),
        })
    return in_maps


def assemble(results):
    out = np.zeros((B, C, H, W), np.float32)
    for core in range(8):
        b, half = core // 2, core % 2
        out[b, :, 32 * half:32 * half + 32, :] = \
            results[core]["out"].reshape(C, 32, W)
    return out


_NC_CACHE = {}


def _get_nc(reps=1):
    if reps not in _NC_CACHE:
        _NC_CACHE[reps] = build_nc(reps)
    return _NC_CACHE[reps]


def run(x, w_qkv, w_pe, w_proj, reps=1, **spmd_kwargs):
    nc = _get_nc(reps)
    in_maps = host_prep(x, w_qkv, w_pe, w_proj)
    res = run_bass_kernel_spmd(nc, in_maps, core_ids=list(range(8)),
                               **spmd_kwargs)
    return assemble(res.results), res


def kernel(x, w_qkv, w_pe, w_proj):
    out, _ = run(x, w_qkv, w_pe, w_proj)
    return out
